# revision 13
# baseline (speedup 1.0000x reference)
"""Trainium2 Bass kernel for nn_DampedIMEX1Layer (v2).

The reference's per-step 2x2 transition M is constant over time, so the
associative scan is a constant-coefficient linear recurrence
    z_t = M z_{t-1} + c * u_t,  u = x @ Bc^T (complex),  y_t = z_t[1]
    out = Re(ys @ Cc^T) + D * x

Device algorithm (one batch element per core, 8 cores):
  chunks of T=8 timesteps, C=1024 chunks, level-1 blocks of Q1=8 chunks
  (B1=128 blocks). HOST re-orders x so every matmul rhs is contiguous:
  chunk q = t*B1 + b (plane-major), x laid out [half][lag j][chunk q].
  - extraction (PE): g streams (k, re/im) per chunk via 8 accumulating
    matmuls per (half, stream); psum -> s[k] tiles interleaved [q][ri].
  - chain (DVE, fp32): level-1 scan over planes (contiguous 256-el STT),
    Hillis-Steele scan over the 128 block-ends, then per-plane fixup.
  - local + injection (PE): per (half, tau) one psum tile accumulates
    tau+1 lag matmuls (Phi_s) plus 4 injection matmuls (Psi) reading
    bf16 shifted-state tiles; ACT drains psum -> out (bf16).
All matmuls are bf16 (host pre-rounded) with fp32 psum accumulation.
Host un-permutes the output and casts to fp32.
"""
import numpy as np
import ml_dtypes

import concourse.bacc as bacc
import concourse.mybir as mybir
from concourse.tile import TileContext
from concourse import bass_utils

P, H, L, BSZ = 128, 128, 8192, 8
T = 8
C = L // T           # 1024
Q1 = 8
B1 = C // Q1         # 128
NH = 2
CH = C // NH         # 512 chunks per half
PL = 2 * B1          # 256 floats per plane (b, ri interleaved)
NCOEF = 4 + 7 * 4 + 7 * 4

F32 = mybir.dt.float32
BF16 = mybir.dt.bfloat16
BF = ml_dtypes.bfloat16


def _bf16r(a):
    return np.ascontiguousarray(np.asarray(a, np.float64).astype(np.float32)).astype(BF)


def _host_params(A_diag, G_diag, dt):
    f = np.float32
    dt_s = (1.0 / (1.0 + np.exp(-dt.astype(np.float64)))).astype(f)
    A = np.maximum(A_diag.astype(f), f(0.0))
    G = np.maximum(G_diag.astype(f), f(0.0))
    dt2 = np.maximum(dt_s * dt_s, f(1e-6))
    s = np.sqrt(f(1.0) + dt_s * G)
    A_low = (f(2.0) + dt_s * G - f(2.0) * s) / dt2
    A_high = (f(2.0) + dt_s * G + f(2.0) * s) / dt2
    A_fin = A_low + np.maximum(A - A_low, f(0)) - np.maximum(A - A_high, f(0))
    S = f(1.0) + dt_s * G
    M = np.stack([np.stack([f(1.0) / S, -(dt_s / S) * A_fin], -1),
                  np.stack([dt_s / S, f(1.0) - (dt_s * dt_s / S) * A_fin], -1)], -2)
    c = np.stack([dt_s / S, dt_s * dt_s / S], -1)
    return M.astype(np.float64), c.astype(np.float64)


def _mat_powers(M, n):
    out = np.empty((n, P, 2, 2))
    out[0] = np.eye(2)[None]
    for i in range(1, n):
        out[i] = np.einsum('pij,pjk->pik', M, out[i - 1])
    return out


def _flat22(A):  # (P,2,2) -> (P,4) [a00,a01,a10,a11]
    return np.stack([A[:, 0, 0], A[:, 0, 1], A[:, 1, 0], A[:, 1, 1]], 1).astype(np.float32)


def _host_weights(A_diag, G_diag, dt, B, C_, D):
    M, c = _host_params(A_diag, G_diag, dt)
    Bre = B[..., 0].astype(np.float64)
    Bim = B[..., 1].astype(np.float64)
    Cre = C_[..., 0].astype(np.float64)
    Cim = C_[..., 1].astype(np.float64)

    Mp = _mat_powers(M, T + 1)
    K = np.einsum('spij,pj->spi', Mp[:T], c)[:, :, 1]     # (T,P)
    MT = Mp[T]

    phi = np.empty((H, T * H), BF)                        # lhsT slot s
    for s in range(T):
        Phi = (Cre * K[s]) @ Bre - (Cim * K[s]) @ Bim
        if s == 0:
            Phi = Phi + np.diag(D.astype(np.float64))
        phi[:, s * H:(s + 1) * H] = _bf16r(Phi.T)

    w = np.einsum('spij,pj->spi', Mp[:T][::-1], c)        # w[j] = M^{T-1-j} c
    vt = np.empty((H, 4 * T * P), BF)                     # slot (k*2+ri)*T+j
    for j in range(T):
        for k in range(2):
            vt[:, ((k * 2 + 0) * T + j) * P:((k * 2 + 0) * T + j + 1) * P] = \
                _bf16r(Bre.T * w[j, :, k])
            vt[:, ((k * 2 + 1) * T + j) * P:((k * 2 + 1) * T + j + 1) * P] = \
                _bf16r(Bim.T * w[j, :, k])

    Winj = Mp[1:T + 1][:, :, 1, :]                        # (T,P,2)
    psi = np.empty((P, 4 * T * H), BF)                    # slot (tau*2+k)*2+ri
    for t in range(T):
        for k in range(2):
            psi[:, ((t * 2 + k) * 2 + 0) * H:((t * 2 + k) * 2 + 1) * H] = \
                _bf16r(Cre.T * Winj[t, :, k][:, None])
            psi[:, ((t * 2 + k) * 2 + 1) * H:((t * 2 + k) * 2 + 2) * H] = \
                _bf16r(-Cim.T * Winj[t, :, k][:, None])

    A8 = _mat_powers(MT, Q1 + 1)[Q1]
    hs = []
    Pk = A8
    for _ in range(7):
        hs.append(_flat22(Pk))
        Pk = np.einsum('pij,pjk->pik', Pk, Pk)
    A1p = _mat_powers(MT, Q1 + 1)
    fix = [_flat22(A1p[t + 1]) for t in range(Q1 - 1)]
    coef = np.concatenate([_flat22(MT)] + hs + fix, 1).astype(np.float32)
    return dict(phi=phi, vt=vt, psi=psi, coef=np.ascontiguousarray(coef))


def _permute_x(x):
    """(BSZ, L, H) fp32 -> (BSZ, H, L) bf16, layout [half][j][q]."""
    xt = x.transpose(0, 2, 1).reshape(BSZ, H, C, T)
    q = np.arange(C)
    i_of_q = (q % B1) * Q1 + q // B1
    xp = xt[:, :, i_of_q, :].transpose(0, 1, 3, 2)        # (B,H,T,C) [j][q]
    xp = xp.reshape(BSZ, H, T, NH, CH).transpose(0, 1, 3, 2, 4)
    return np.ascontiguousarray(xp.reshape(BSZ, H, L)).astype(BF)


def _build_nc():
    nc = bacc.Bacc("TRN2", target_bir_lowering=False, debug=False, num_devices=8)
    Alu = mybir.AluOpType

    x_d = nc.dram_tensor("x", (P, L), BF16, kind="ExternalInput").ap()
    vt_d = nc.dram_tensor("vt", (H, 4 * T * P), BF16, kind="ExternalInput").ap()
    phi_d = nc.dram_tensor("phi", (H, T * H), BF16, kind="ExternalInput").ap()
    psi_d = nc.dram_tensor("psi", (P, 4 * T * H), BF16, kind="ExternalInput").ap()
    coef_d = nc.dram_tensor("coef", (P, NCOEF), F32, kind="ExternalInput").ap()
    out_d = nc.dram_tensor("out", (P, L), BF16, kind="ExternalOutput").ap()

    with TileContext(nc) as tc:
        with (
            tc.tile_pool(name="const", bufs=1) as cp,
            tc.tile_pool(name="ps", bufs=8, space="PSUM") as ps,
        ):
            # ---------- DMA in ----------
            # vt is laid out per-stream ((k*2+ri)*T+j); the first ext stream
            # only needs the first quarter, so chunk the vt DMA.
            vt_sb = cp.tile([H, 4 * T * P], BF16, tag="vt")
            nc.sync.dma_start(vt_sb[:, 0:T * P], vt_d[:, 0:T * P])
            x_sb = cp.tile([P, L], BF16, tag="x")
            for i in range(8):
                sl = slice(i * 512, (i + 1) * 512)
                nc.sync.dma_start(x_sb[:, sl], x_d[:, sl])
            for cq in range(1, 4):
                sl = slice(cq * T * P, (cq + 1) * T * P)
                nc.sync.dma_start(vt_sb[:, sl], vt_d[:, sl])
            coef_sb = cp.tile([P, NCOEF], F32, tag="coef")
            nc.sync.dma_start(coef_sb[:], coef_d)
            for i in range(8, 16):
                sl = slice(i * 512, (i + 1) * 512)
                nc.sync.dma_start(x_sb[:, sl], x_d[:, sl])
            phi_sb = cp.tile([H, T * H], BF16, tag="phi")
            nc.sync.dma_start(phi_sb[:], phi_d)
            psi_sb = cp.tile([P, 4 * T * H], BF16, tag="psi")
            nc.sync.dma_start(psi_sb[:], psi_d)

            s0 = cp.tile([P, 2 * C], BF16, tag="s0")
            s1 = cp.tile([P, 2 * C], BF16, tag="s1")
            st = {0: s0, 1: s1}
            sh = {}
            for k in range(2):
                for ri in range(2):
                    sh[(k, ri)] = cp.tile([P, C], BF16, tag=f"sh{k}{ri}", name=f"sh{k}{ri}")
            out_sb = cp.tile([P, L], BF16, tag="out")
            bcA = {k: cp.tile([P, PL], BF16, tag=f"bcA{k}", name=f"bcA{k}") for k in range(2)}
            bcB = {k: cp.tile([P, PL], BF16, tag=f"bcB{k}", name=f"bcB{k}") for k in range(2)}
            tm0 = cp.tile([P, PL], BF16, tag="tm0")
            tm1 = cp.tile([P, PL], BF16, tag="tm1")

            def cc(i):  # coef column
                return coef_sb[:, i:i + 1]

            # zero the q=0 column of sh (b=0, plane 0) before bc-part copies
            for k in range(2):
                for ri in range(2):
                    nc.vector.memset(sh[(k, ri)][:, 0:2].bitcast(F32), 0.0)

            # All chain STT ops run on DVE (bf16 for 2x throughput); the
            # copies run on the otherwise-idle ACT engine so they stay off
            # the DVE critical path.

            # ---------- extraction (PE) + interleave copies (DVE/GPSIMD) ----
            def ext_half(h):
                for k in range(2):
                    for ri in range(2):
                        pt = ps.tile([P, CH], F32, tag="ps", name="pt")
                        for j in range(T):
                            qidx = (k * 2 + ri) * T + j
                            nc.tensor.matmul(
                                pt[:], vt_sb[:, qidx * P:(qidx + 1) * P],
                                x_sb[:, h * 8 * CH + j * CH:(h * 8 + j + 1) * CH],
                                start=(j == 0), stop=(j == T - 1))
                        dst = st[k][:].rearrange("p (c r) -> p c r", r=2)[
                            :, h * CH:(h + 1) * CH, ri]
                        nc.scalar.copy(dst, pt[:])

            def level1(tlo, thi):
                for t in range(tlo, thi):
                    pp = slice((t - 1) * PL, t * PL)
                    pc = slice(t * PL, (t + 1) * PL)
                    nc.vector.scalar_tensor_tensor(
                        tm0[:], s1[:, pp], cc(1), s0[:, pc], Alu.mult, Alu.add)
                    nc.vector.scalar_tensor_tensor(
                        tm1[:], s0[:, pp], cc(2), s1[:, pc], Alu.mult, Alu.add)
                    nc.vector.scalar_tensor_tensor(
                        s0[:, pc], s0[:, pp], cc(0), tm0[:], Alu.mult, Alu.add)
                    nc.vector.scalar_tensor_tensor(
                        s1[:, pc], s1[:, pp], cc(3), tm1[:], Alu.mult, Alu.add)

            ext_half(0)
            ext_half(1)
            level1(1, 4)
            level1(4, Q1)

            # ---------- Hillis-Steele over block-ends (DVE/GPSIMD) ----------
            nc.scalar.copy(bcA[0][:], s0[:, (Q1 - 1) * PL:Q1 * PL])
            nc.scalar.copy(bcA[1][:], s1[:, (Q1 - 1) * PL:Q1 * PL])
            cur, nxt = bcA, bcB
            for lv in range(7):
                d2 = 2 * (1 << lv)
                cb = 4 + lv * 4
                n = PL - d2
                lo = slice(0, n)
                hi = slice(d2, PL)
                nc.vector.scalar_tensor_tensor(
                    tm0[:, 0:n], cur[1][:, lo], cc(cb + 1), cur[0][:, hi],
                    Alu.mult, Alu.add)
                nc.vector.scalar_tensor_tensor(
                    tm1[:, 0:n], cur[0][:, lo], cc(cb + 2), cur[1][:, hi],
                    Alu.mult, Alu.add)
                nc.vector.scalar_tensor_tensor(
                    nxt[0][:, hi], cur[0][:, lo], cc(cb), tm0[:, 0:n],
                    Alu.mult, Alu.add)
                nc.vector.scalar_tensor_tensor(
                    nxt[1][:, hi], cur[1][:, lo], cc(cb + 3), tm1[:, 0:n],
                    Alu.mult, Alu.add)
                nc.scalar.copy(nxt[0][:, 0:d2], cur[0][:, 0:d2])
                nc.scalar.copy(nxt[1][:, 0:d2], cur[1][:, 0:d2])
                cur, nxt = nxt, cur
            bc = cur  # final block-end states

            # sh bc-part (ACT): sh[k][ri][:, 1:B1] = bc[k][b=0..126][ri]
            for k in range(2):
                for ri in range(2):
                    src = bc[k][:].rearrange("p (b r) -> p b r", r=2)[:, 0:B1 - 1, ri]
                    nc.scalar.copy(sh[(k, ri)][:, 1:B1], src)

            # ------- level-1 fixup (DVE/GPSIMD) + sh plane copies (ACT) -----
            for t in range(Q1 - 1):
                cb = 4 + 28 + t * 4
                pc0 = s0[:, t * PL + 2:(t + 1) * PL]
                pc1 = s1[:, t * PL + 2:(t + 1) * PL]
                bp = slice(0, PL - 2)
                nc.vector.scalar_tensor_tensor(
                    tm0[:, 0:PL - 2], bc[1][:, bp], cc(cb + 1), pc0,
                    Alu.mult, Alu.add)
                nc.vector.scalar_tensor_tensor(
                    tm1[:, 0:PL - 2], bc[0][:, bp], cc(cb + 2), pc1,
                    Alu.mult, Alu.add)
                nc.vector.scalar_tensor_tensor(
                    pc0, bc[0][:, bp], cc(cb), tm0[:, 0:PL - 2],
                    Alu.mult, Alu.add)
                nc.vector.scalar_tensor_tensor(
                    pc1, bc[1][:, bp], cc(cb + 3), tm1[:, 0:PL - 2],
                    Alu.mult, Alu.add)
                # sh plane part: out plane t+1 reads fixed s plane t
                for k in range(2):
                    for ri in range(2):
                        src = st[k][:].rearrange("p (b r) -> p b r", r=2)[
                            :, t * B1:(t + 1) * B1, ri]
                        nc.scalar.copy(
                            sh[(k, ri)][:, (t + 1) * B1:(t + 2) * B1], src)

            # ---------- local + injection (PE), drain (ACT) ----------
            for h in range(NH):
                for tau in range(T):
                    pt = ps.tile([P, CH], F32, tag="ps", name="pt")
                    for sg in range(tau + 1):
                        nc.tensor.matmul(
                            pt[:], phi_sb[:, sg * H:(sg + 1) * H],
                            x_sb[:, h * 8 * CH + (tau - sg) * CH:
                                 h * 8 * CH + (tau - sg + 1) * CH],
                            start=(sg == 0), stop=False)
                    mi = 0
                    for k in range(2):
                        for ri in range(2):
                            qidx = (tau * 2 + k) * 2 + ri
                            nc.tensor.matmul(
                                pt[:], psi_sb[:, qidx * H:(qidx + 1) * H],
                                sh[(k, ri)][:, h * CH:(h + 1) * CH],
                                start=False, stop=(mi == 3))
                            mi += 1
                    sl = slice((h * 8 + tau) * CH, (h * 8 + tau + 1) * CH)
                    nc.scalar.copy(out_sb[:, sl], pt[:])
                    nc.sync.dma_start(out_d[:, sl], out_sb[:, sl])

    nc.compile()
    return nc


_NC_CACHE = None


def kernel(x, A_diag, G_diag, dt, B, C, D):
    global _NC_CACHE
    x = np.asarray(x, dtype=np.float32)
    wts = _host_weights(np.asarray(A_diag, np.float32), np.asarray(G_diag, np.float32),
                        np.asarray(dt, np.float32), np.asarray(B, np.float32),
                        np.asarray(C, np.float32), np.asarray(D, np.float32))
    xp = _permute_x(x)

    if _NC_CACHE is None:
        _NC_CACHE = _build_nc()
    nc = _NC_CACHE

    common = {"vt": wts["vt"], "phi": wts["phi"], "psi": wts["psi"],
              "coef": wts["coef"]}
    in_maps = [dict(common, x=xp[b]) for b in range(BSZ)]
    res = bass_utils.run_bass_kernel_spmd(
        nc, in_maps, core_ids=list(range(BSZ)), trace=False)
    nchunk = L // T
    q = np.arange(nchunk)
    i_of_q = (q % B1) * Q1 + q // B1
    q_of_i = np.empty(nchunk, np.int64)
    q_of_i[i_of_q] = q
    outs = []
    for b in range(BSZ):
        o = res.results[b]["out"].astype(np.float32)      # (H, L) [half][tau][q]
        ot = o.reshape(H, NH, T, CH).transpose(0, 2, 1, 3).reshape(H, T, nchunk)
        ot = ot[:, :, q_of_i]                             # [tau][chunk i]
        outs.append(ot.transpose(0, 2, 1).reshape(H, L).T)  # (L, H)
    return np.ascontiguousarray(np.stack(outs, 0))


# revision 16
# speedup vs baseline: 1.0958x; 1.0958x over previous
"""Trainium2 Bass kernel for nn_DampedIMEX1Layer (v2).

The reference's per-step 2x2 transition M is constant over time, so the
associative scan is a constant-coefficient linear recurrence
    z_t = M z_{t-1} + c * u_t,  u = x @ Bc^T (complex),  y_t = z_t[1]
    out = Re(ys @ Cc^T) + D * x

Device algorithm (one batch element per core, 8 cores):
  chunks of T=8 timesteps, C=1024 chunks, level-1 blocks of Q1=8 chunks
  (B1=128 blocks). HOST re-orders x so every matmul rhs is contiguous:
  chunk q = t*B1 + b (plane-major), x laid out [half][lag j][chunk q].
  - extraction (PE): g streams (k, re/im) per chunk via 8 accumulating
    matmuls per (half, stream); psum -> s[k] tiles interleaved [q][ri].
  - chain (DVE, fp32): level-1 scan over planes (contiguous 256-el STT),
    Hillis-Steele scan over the 128 block-ends, then per-plane fixup.
  - local + injection (PE): per (half, tau) one psum tile accumulates
    tau+1 lag matmuls (Phi_s) plus 4 injection matmuls (Psi) reading
    bf16 shifted-state tiles; ACT drains psum -> out (bf16).
All matmuls are bf16 (host pre-rounded) with fp32 psum accumulation.
Host un-permutes the output and casts to fp32.
"""
import numpy as np
import ml_dtypes

import concourse.bacc as bacc
import concourse.mybir as mybir
from concourse.tile import TileContext
from concourse import bass_utils

P, H, L, BSZ = 128, 128, 8192, 8
T = 8
C = L // T           # 1024
Q1 = 8
B1 = C // Q1         # 128
NH = 2
CH = C // NH         # 512 chunks per half
PL = 2 * B1          # 256 floats per plane (b, ri interleaved)
NCOEF = 4 + 7 * 4 + 7 * 4

F32 = mybir.dt.float32
BF16 = mybir.dt.bfloat16
BF = ml_dtypes.bfloat16


def _bf16r(a):
    return np.ascontiguousarray(np.asarray(a, np.float64).astype(np.float32)).astype(BF)


def _host_params(A_diag, G_diag, dt):
    f = np.float32
    dt_s = (1.0 / (1.0 + np.exp(-dt.astype(np.float64)))).astype(f)
    A = np.maximum(A_diag.astype(f), f(0.0))
    G = np.maximum(G_diag.astype(f), f(0.0))
    dt2 = np.maximum(dt_s * dt_s, f(1e-6))
    s = np.sqrt(f(1.0) + dt_s * G)
    A_low = (f(2.0) + dt_s * G - f(2.0) * s) / dt2
    A_high = (f(2.0) + dt_s * G + f(2.0) * s) / dt2
    A_fin = A_low + np.maximum(A - A_low, f(0)) - np.maximum(A - A_high, f(0))
    S = f(1.0) + dt_s * G
    M = np.stack([np.stack([f(1.0) / S, -(dt_s / S) * A_fin], -1),
                  np.stack([dt_s / S, f(1.0) - (dt_s * dt_s / S) * A_fin], -1)], -2)
    c = np.stack([dt_s / S, dt_s * dt_s / S], -1)
    return M.astype(np.float64), c.astype(np.float64)


def _mat_powers(M, n):
    out = np.empty((n, P, 2, 2))
    out[0] = np.eye(2)[None]
    for i in range(1, n):
        out[i] = np.einsum('pij,pjk->pik', M, out[i - 1])
    return out


def _flat22(A):  # (P,2,2) -> (P,4) [a00,a01,a10,a11]
    return np.stack([A[:, 0, 0], A[:, 0, 1], A[:, 1, 0], A[:, 1, 1]], 1).astype(np.float32)


def _host_weights(A_diag, G_diag, dt, B, C_, D):
    M, c = _host_params(A_diag, G_diag, dt)
    Bre = B[..., 0].astype(np.float64)
    Bim = B[..., 1].astype(np.float64)
    Cre = C_[..., 0].astype(np.float64)
    Cim = C_[..., 1].astype(np.float64)

    Mp = _mat_powers(M, T + 1)
    K = np.einsum('spij,pj->spi', Mp[:T], c)[:, :, 1]     # (T,P)
    MT = Mp[T]

    phi = np.empty((H, T * H), BF)                        # lhsT slot s
    for s in range(T):
        Phi = (Cre * K[s]) @ Bre - (Cim * K[s]) @ Bim
        if s == 0:
            Phi = Phi + np.diag(D.astype(np.float64))
        phi[:, s * H:(s + 1) * H] = _bf16r(Phi.T)

    w = np.einsum('spij,pj->spi', Mp[:T][::-1], c)        # w[j] = M^{T-1-j} c
    vt = np.empty((H, 4 * T * P), BF)                     # slot (k*2+ri)*T+j
    for j in range(T):
        for k in range(2):
            vt[:, ((k * 2 + 0) * T + j) * P:((k * 2 + 0) * T + j + 1) * P] = \
                _bf16r(Bre.T * w[j, :, k])
            vt[:, ((k * 2 + 1) * T + j) * P:((k * 2 + 1) * T + j + 1) * P] = \
                _bf16r(Bim.T * w[j, :, k])

    Winj = Mp[1:T + 1][:, :, 1, :]                        # (T,P,2)
    psi = np.empty((P, 4 * T * H), BF)                    # slot (tau*2+k)*2+ri
    for t in range(T):
        for k in range(2):
            psi[:, ((t * 2 + k) * 2 + 0) * H:((t * 2 + k) * 2 + 1) * H] = \
                _bf16r(Cre.T * Winj[t, :, k][:, None])
            psi[:, ((t * 2 + k) * 2 + 1) * H:((t * 2 + k) * 2 + 2) * H] = \
                _bf16r(-Cim.T * Winj[t, :, k][:, None])

    A8 = _mat_powers(MT, Q1 + 1)[Q1]
    hs = []
    Pk = A8
    for _ in range(7):
        hs.append(_flat22(Pk))
        Pk = np.einsum('pij,pjk->pik', Pk, Pk)
    A1p = _mat_powers(MT, Q1 + 1)
    fix = [_flat22(A1p[t + 1]) for t in range(Q1 - 1)]
    coef = np.concatenate([_flat22(MT)] + hs + fix, 1).astype(np.float32)
    return dict(phi=phi, vt=vt, psi=psi, coef=np.ascontiguousarray(coef))


def _permute_x(x):
    """(BSZ, L, H) fp32 -> (BSZ, H, L) bf16, layout [half][j][q]."""
    xt = x.transpose(0, 2, 1).reshape(BSZ, H, C, T)
    q = np.arange(C)
    i_of_q = (q % B1) * Q1 + q // B1
    xp = xt[:, :, i_of_q, :].transpose(0, 1, 3, 2)        # (B,H,T,C) [j][q]
    xp = xp.reshape(BSZ, H, T, NH, CH).transpose(0, 1, 3, 2, 4)
    return np.ascontiguousarray(xp.reshape(BSZ, H, L)).astype(BF)


def _build_nc():
    nc = bacc.Bacc("TRN2", target_bir_lowering=False, debug=False, num_devices=8)
    Alu = mybir.AluOpType

    x_d = nc.dram_tensor("x", (P, L), BF16, kind="ExternalInput").ap()
    vt_d = nc.dram_tensor("vt", (H, 4 * T * P), BF16, kind="ExternalInput").ap()
    phi_d = nc.dram_tensor("phi", (H, T * H), BF16, kind="ExternalInput").ap()
    psi_d = nc.dram_tensor("psi", (P, 4 * T * H), BF16, kind="ExternalInput").ap()
    coef_d = nc.dram_tensor("coef", (P, NCOEF), F32, kind="ExternalInput").ap()
    out_d = nc.dram_tensor("out", (P, L), BF16, kind="ExternalOutput").ap()

    with TileContext(nc) as tc:
        with (
            tc.tile_pool(name="const", bufs=1) as cp,
            tc.tile_pool(name="ps", bufs=6, space="PSUM") as ps,
        ):
            # ---------- DMA in ----------
            # vt is laid out per-stream ((k*2+ri)*T+j); the first ext stream
            # only needs the first quarter, so chunk the vt DMA.
            vt_sb = cp.tile([H, 4 * T * P], BF16, tag="vt")
            nc.sync.dma_start(vt_sb[:, 0:T * P], vt_d[:, 0:T * P])
            x_sb = cp.tile([P, L], BF16, tag="x")
            for i in range(8):
                sl = slice(i * 512, (i + 1) * 512)
                nc.sync.dma_start(x_sb[:, sl], x_d[:, sl])
            for cq in range(1, 4):
                sl = slice(cq * T * P, (cq + 1) * T * P)
                nc.sync.dma_start(vt_sb[:, sl], vt_d[:, sl])
            coef_sb = cp.tile([P, NCOEF], F32, tag="coef")
            nc.sync.dma_start(coef_sb[:], coef_d)
            for i in range(8, 16):
                sl = slice(i * 512, (i + 1) * 512)
                nc.sync.dma_start(x_sb[:, sl], x_d[:, sl])
            phi_sb = cp.tile([H, T * H], BF16, tag="phi")
            nc.sync.dma_start(phi_sb[:], phi_d)
            psi_sb = cp.tile([P, 4 * T * H], BF16, tag="psi")
            nc.sync.dma_start(psi_sb[:], psi_d)

            s0 = cp.tile([P, 2 * C], BF16, tag="s0")
            s1 = cp.tile([P, 2 * C], BF16, tag="s1")
            st = {0: s0, 1: s1}
            sh = {}
            for k in range(2):
                for ri in range(2):
                    sh[(k, ri)] = cp.tile([P, C], BF16, tag=f"sh{k}{ri}", name=f"sh{k}{ri}")
            out_sb = cp.tile([P, L], BF16, tag="out")
            bcA = {k: cp.tile([P, PL], BF16, tag=f"bcA{k}", name=f"bcA{k}") for k in range(2)}
            bcB = {k: cp.tile([P, PL], BF16, tag=f"bcB{k}", name=f"bcB{k}") for k in range(2)}
            tm0 = cp.tile([P, PL], BF16, tag="tm0")
            tm1 = cp.tile([P, PL], BF16, tag="tm1")

            def cc(i):  # coef column
                return coef_sb[:, i:i + 1]

            # zero the q=0 column of sh (b=0, plane 0) before bc-part copies
            for k in range(2):
                for ri in range(2):
                    nc.vector.memset(sh[(k, ri)][:, 0:2].bitcast(F32), 0.0)

            # All chain STT ops run on DVE (bf16 for 2x throughput); the
            # copies run on the otherwise-idle ACT engine so they stay off
            # the DVE critical path.

            # ---------- extraction (PE) + interleave copies (DVE/GPSIMD) ----
            def ext_half(h):
                for k in range(2):
                    for ri in range(2):
                        pt = ps.tile([P, CH], F32, tag="ps", name="pt")
                        for j in range(T):
                            qidx = (k * 2 + ri) * T + j
                            nc.tensor.matmul(
                                pt[:], vt_sb[:, qidx * P:(qidx + 1) * P],
                                x_sb[:, h * 8 * CH + j * CH:(h * 8 + j + 1) * CH],
                                start=(j == 0), stop=(j == T - 1))
                        dst = st[k][:].rearrange("p (c r) -> p c r", r=2)[
                            :, h * CH:(h + 1) * CH, ri]
                        nc.scalar.copy(dst, pt[:])

            def level1(tlo, thi):
                for t in range(tlo, thi):
                    pp = slice((t - 1) * PL, t * PL)
                    pc = slice(t * PL, (t + 1) * PL)
                    nc.vector.scalar_tensor_tensor(
                        tm0[:], s1[:, pp], cc(1), s0[:, pc], Alu.mult, Alu.add)
                    nc.vector.scalar_tensor_tensor(
                        tm1[:], s0[:, pp], cc(2), s1[:, pc], Alu.mult, Alu.add)
                    nc.vector.scalar_tensor_tensor(
                        s0[:, pc], s0[:, pp], cc(0), tm0[:], Alu.mult, Alu.add)
                    nc.vector.scalar_tensor_tensor(
                        s1[:, pc], s1[:, pp], cc(3), tm1[:], Alu.mult, Alu.add)

            ext_half(0)
            ext_half(1)
            level1(1, 4)
            level1(4, Q1)

            # ---------- Hillis-Steele over block-ends ----------
            nc.vector.tensor_copy(bcA[0][:], s0[:, (Q1 - 1) * PL:Q1 * PL])
            nc.vector.tensor_copy(bcA[1][:], s1[:, (Q1 - 1) * PL:Q1 * PL])
            cur, nxt = bcA, bcB
            for lv in range(7):
                d2 = 2 * (1 << lv)
                cb = 4 + lv * 4
                n = PL - d2
                lo = slice(0, n)
                hi = slice(d2, PL)
                nc.vector.scalar_tensor_tensor(
                    tm0[:, 0:n], cur[1][:, lo], cc(cb + 1), cur[0][:, hi],
                    Alu.mult, Alu.add)
                nc.vector.scalar_tensor_tensor(
                    tm1[:, 0:n], cur[0][:, lo], cc(cb + 2), cur[1][:, hi],
                    Alu.mult, Alu.add)
                nc.vector.scalar_tensor_tensor(
                    nxt[0][:, hi], cur[0][:, lo], cc(cb), tm0[:, 0:n],
                    Alu.mult, Alu.add)
                nc.vector.scalar_tensor_tensor(
                    nxt[1][:, hi], cur[1][:, lo], cc(cb + 3), tm1[:, 0:n],
                    Alu.mult, Alu.add)
                nc.scalar.copy(nxt[0][:, 0:d2], cur[0][:, 0:d2])
                nc.scalar.copy(nxt[1][:, 0:d2], cur[1][:, 0:d2])
                cur, nxt = nxt, cur
            bc = cur  # final block-end states

            # sh bc-part (ACT): sh[k][ri][:, 1:B1] = bc[k][b=0..126][ri]
            for k in range(2):
                for ri in range(2):
                    src = bc[k][:].rearrange("p (b r) -> p b r", r=2)[:, 0:B1 - 1, ri]
                    nc.scalar.copy(sh[(k, ri)][:, 1:B1], src)

            # ------- level-1 fixup (DVE/GPSIMD) + sh plane copies (ACT) -----
            for t in range(Q1 - 1):
                cb = 4 + 28 + t * 4
                pc0 = s0[:, t * PL + 2:(t + 1) * PL]
                pc1 = s1[:, t * PL + 2:(t + 1) * PL]
                bp = slice(0, PL - 2)
                nc.vector.scalar_tensor_tensor(
                    tm0[:, 0:PL - 2], bc[1][:, bp], cc(cb + 1), pc0,
                    Alu.mult, Alu.add)
                nc.vector.scalar_tensor_tensor(
                    tm1[:, 0:PL - 2], bc[0][:, bp], cc(cb + 2), pc1,
                    Alu.mult, Alu.add)
                nc.vector.scalar_tensor_tensor(
                    pc0, bc[0][:, bp], cc(cb), tm0[:, 0:PL - 2],
                    Alu.mult, Alu.add)
                nc.vector.scalar_tensor_tensor(
                    pc1, bc[1][:, bp], cc(cb + 3), tm1[:, 0:PL - 2],
                    Alu.mult, Alu.add)
                # sh plane part: out plane t+1 reads fixed s plane t
                for k in range(2):
                    for ri in range(2):
                        src = st[k][:].rearrange("p (b r) -> p b r", r=2)[
                            :, t * B1:(t + 1) * B1, ri]
                        nc.scalar.copy(
                            sh[(k, ri)][:, (t + 1) * B1:(t + 2) * B1], src)

            # ---------- local (PE, independent of the chain) ----------
            # Both halves run before the chain finishes (psum tiles rotate
            # through the pool); local-only results park in oloc so the PE
            # never idles waiting on psum banks held for injection.
            oloc = cp.tile([P, L], F32, tag="oloc")
            for h in range(NH):
                for tau in range(T):
                    pt = ps.tile([P, CH], F32, tag="ps", name="pt")
                    for sg in range(tau + 1):
                        nc.tensor.matmul(
                            pt[:], phi_sb[:, sg * H:(sg + 1) * H],
                            x_sb[:, h * 8 * CH + (tau - sg) * CH:
                                 h * 8 * CH + (tau - sg + 1) * CH],
                            start=(sg == 0), stop=(sg == tau))
                    sl = slice((h * 8 + tau) * CH, (h * 8 + tau + 1) * CH)
                    nc.scalar.copy(oloc[:, sl], pt[:])

            # ---------- injection (PE) + assembly (DVE) + out DMA ----------
            for h in range(NH):
                for tau in range(T):
                    pi = ps.tile([P, CH], F32, tag="ps", name="pi")
                    mi = 0
                    for k in range(2):
                        for ri in range(2):
                            qidx = (tau * 2 + k) * 2 + ri
                            nc.tensor.matmul(
                                pi[:], psi_sb[:, qidx * H:(qidx + 1) * H],
                                sh[(k, ri)][:, h * CH:(h + 1) * CH],
                                start=(mi == 0), stop=(mi == 3))
                            mi += 1
                    sl = slice((h * 8 + tau) * CH, (h * 8 + tau + 1) * CH)
                    nc.vector.tensor_tensor(out_sb[:, sl], pi[:], oloc[:, sl],
                                            Alu.add)
                    nc.sync.dma_start(out_d[:, sl], out_sb[:, sl])

    nc.compile()
    return nc


_NC_CACHE = None


def kernel(x, A_diag, G_diag, dt, B, C, D):
    global _NC_CACHE
    x = np.asarray(x, dtype=np.float32)
    wts = _host_weights(np.asarray(A_diag, np.float32), np.asarray(G_diag, np.float32),
                        np.asarray(dt, np.float32), np.asarray(B, np.float32),
                        np.asarray(C, np.float32), np.asarray(D, np.float32))
    xp = _permute_x(x)

    if _NC_CACHE is None:
        _NC_CACHE = _build_nc()
    nc = _NC_CACHE

    common = {"vt": wts["vt"], "phi": wts["phi"], "psi": wts["psi"],
              "coef": wts["coef"]}
    in_maps = [dict(common, x=xp[b]) for b in range(BSZ)]
    res = bass_utils.run_bass_kernel_spmd(
        nc, in_maps, core_ids=list(range(BSZ)), trace=False)
    nchunk = L // T
    q = np.arange(nchunk)
    i_of_q = (q % B1) * Q1 + q // B1
    q_of_i = np.empty(nchunk, np.int64)
    q_of_i[i_of_q] = q
    outs = []
    for b in range(BSZ):
        o = res.results[b]["out"].astype(np.float32)      # (H, L) [half][tau][q]
        ot = o.reshape(H, NH, T, CH).transpose(0, 2, 1, 3).reshape(H, T, nchunk)
        ot = ot[:, :, q_of_i]                             # [tau][chunk i]
        outs.append(ot.transpose(0, 2, 1).reshape(H, L).T)  # (L, H)
    return np.ascontiguousarray(np.stack(outs, 0))
